# revision 6
# baseline (speedup 1.0000x reference)
"""Trainium2 Bass kernel for nn_BaseLayer (MoE routing, 8 experts).

Strategy (expert-parallel, per the sharding hint):
  * Host computes the router exactly as the reference does (token-expert
    affinities + argmax + sigmoid gate) with jax-on-CPU so the assignment
    bit-matches the reference, then sorts tokens by expert.  In Bass all
    collectives must be compile-time static, so the dynamic
    dispatch/combine (all_to_all with runtime split sizes) is realized by
    the host sharding step: core e receives expert e's tokens, padded to a
    common capacity C so that a single NEFF runs SPMD on all 8 cores.
  * Each core runs the heavy part on device: LayerNorm -> FF1(+bias,relu)
    -> FF2 -> residual + sigmoid-gated combine, with the expert's weights
    resident in SBUF as bf16 and all matmuls on the PE array.
  * ln_g / ln_b are folded into w1 / b1 on the host (exact for the actual
    inputs where ln_g=1, ln_b=0); b2 is applied on the host during
    unsharding (exact for the actual inputs where b2=0).

The output permutation is the inverse of the sort, so the final output is
independent of sort order; only the argmax assignment must match the
reference, which host-side jax-on-CPU guarantees.
"""

import numpy as np
import ml_dtypes

D = 1024   # embed dim
F = 4096   # ffn dim
E = 8      # experts == cores
P = 128    # partitions
KD = D // P        # 8  k-tiles over D
KF = F // P        # 32 k-tiles over F
GROUP_TILES = 2    # token tiles (of 128) processed per FF1 batch
NW = 4             # weight DMA chunks (consumption-ordered)
EPS = 1e-5


def _routing(x, centroids):
    """Affinity/argmax/alpha exactly like the reference (jax on CPU)."""
    try:
        import jax
        import jax.numpy as jnp

        cpu = jax.devices("cpu")[0]
        with jax.default_device(cpu):
            aff = jnp.asarray(x) @ jnp.asarray(centroids).T
            assign = jnp.argmax(aff, axis=1)
            alpha = jax.nn.sigmoid(
                jnp.take_along_axis(aff, assign[:, None], axis=1)
            )
            return np.asarray(assign), np.asarray(alpha)[:, 0].astype(np.float32)
    except Exception:
        aff = x.astype(np.float32) @ centroids.astype(np.float32).T
        assign = np.argmax(aff, axis=1)
        sel = np.take_along_axis(aff, assign[:, None], axis=1)[:, 0]
        alpha = 1.0 / (1.0 + np.exp(-sel.astype(np.float64)))
        return assign, alpha.astype(np.float32)


def _build(C):
    """Build the per-core Bass program for capacity C (multiple of 128)."""
    import concourse.bacc as bacc
    import concourse.mybir as mybir
    import concourse.tile as tile
    from concourse.masks import make_identity

    f32 = mybir.dt.float32
    bf16 = mybir.dt.bfloat16
    AF = mybir.ActivationFunctionType
    ALU = mybir.AluOpType

    nt = C // P
    groups = []
    t = 0
    while t < nt:
        g = min(GROUP_TILES, nt - t)
        groups.append((t, g))
        t += g

    nc = bacc.Bacc("TRN2", target_bir_lowering=False, debug=False)
    xs_d = nc.dram_tensor("xs", [C, D], f32, kind="ExternalInput").ap()
    al_d = nc.dram_tensor("alphap", [P, nt], f32, kind="ExternalInput").ap()
    w1_d = nc.dram_tensor("w1t", [D, F], bf16, kind="ExternalInput").ap()
    w2_d = nc.dram_tensor("w2t", [F, D], bf16, kind="ExternalInput").ap()
    b1_d = nc.dram_tensor("b1p", [P, KF], f32, kind="ExternalInput").ap()
    out_d = nc.dram_tensor("out", [C, D], f32, kind="ExternalOutput").ap()

    with tile.TileContext(nc) as tc:
        with (
            tc.tile_pool(name="wpool", bufs=1) as wpool,
            tc.tile_pool(name="consts", bufs=1) as consts,
            tc.tile_pool(name="xsp", bufs=4) as xsp,
            tc.tile_pool(name="hp", bufs=2) as hp,
            tc.tile_pool(name="hTp", bufs=2) as hTp,
            tc.tile_pool(name="aTp", bufs=1) as aTp,
            tc.tile_pool(name="statp", bufs=3) as statp,
            tc.tile_pool(name="outp", bufs=3) as outp,
            tc.tile_pool(name="ptrp", bufs=2, space="PSUM") as ptrp,
            tc.tile_pool(name="pap", bufs=2, space="PSUM") as pap,
            tc.tile_pool(name="pyp", bufs=2, space="PSUM") as pyp,
        ):
            ident = consts.tile([P, P], bf16)
            make_identity(nc, ident)
            eps_t = consts.tile([P, 1], f32)
            nc.vector.memset(eps_t, EPS)
            b1_t = consts.tile([P, KF], f32)
            nc.sync.dma_start(b1_t, b1_d)
            al_t = consts.tile([P, nt], f32)
            nc.sync.dma_start(al_t, al_d)

            # Expert weights, resident in SBUF as bf16, DMA'd in NW chunks
            # ordered to match first-group consumption order.
            w1c = []
            w2c = []
            for c in range(NW):
                fw = F // NW
                t1 = wpool.tile([P, KD, fw], bf16, name=f"w1c{c}", tag=f"w1c{c}")
                t2 = wpool.tile([P, KF // NW, D], bf16, name=f"w2c{c}", tag=f"w2c{c}")
                for k in range(KD):
                    nc.sync.dma_start(
                        t1[:, k, :], w1_d[k * P:(k + 1) * P, c * fw:(c + 1) * fw]
                    )
                for kk in range(KF // NW):
                    k2 = c * (KF // NW) + kk
                    nc.sync.dma_start(t2[:, kk, :], w2_d[k2 * P:(k2 + 1) * P, :])
                w1c.append(t1)
                w2c.append(t2)

            for (t0, gt) in groups:
                n = gt * P
                hT = hTp.tile([P, KD, n], bf16, tag="hT")
                xs_tiles = []
                for ti in range(gt):
                    tt = t0 + ti
                    xs_t = xsp.tile([P, D], f32, tag="xs")
                    nc.sync.dma_start(xs_t, xs_d[tt * P:(tt + 1) * P, :])
                    st = statp.tile([P, 2, 6], f32, tag="st")
                    nc.vector.bn_stats(st[:, 0, :], xs_t[:, 0:512])
                    nc.vector.bn_stats(st[:, 1, :], xs_t[:, 512:1024])
                    mv = statp.tile([P, 2], f32, tag="mv")
                    nc.vector.bn_aggr(mv, st)
                    # mv[:,1] := 1/sqrt(var+eps)
                    nc.scalar.activation(
                        mv[:, 1:2], mv[:, 1:2], AF.Sqrt,
                        bias=eps_t[:, 0:1], scale=1.0,
                    )
                    nc.vector.reciprocal(mv[:, 1:2], mv[:, 1:2])
                    h_t = hp.tile([P, D], bf16, tag="h")
                    # h = (x - mean) * rstd
                    nc.vector.tensor_scalar(
                        out=h_t, in0=xs_t,
                        scalar1=mv[:, 0:1], scalar2=mv[:, 1:2],
                        op0=ALU.subtract, op1=ALU.mult,
                    )
                    ptr = ptrp.tile([P, KD, P], bf16, tag="ptr")
                    for k in range(KD):
                        nc.tensor.transpose(
                            ptr[:, k, :], h_t[:, k * P:(k + 1) * P], ident
                        )
                    nc.vector.tensor_copy(hT[:, :, ti * P:(ti + 1) * P], ptr)
                    xs_tiles.append(xs_t)

                # FF1: aT[f, tok] = relu((h @ w1t).T + b1)
                aT = aTp.tile([P, KF, n], bf16, tag="aT")
                for m in range(KF):
                    pa = pap.tile([P, n], f32, tag="pa")
                    cw, mm = divmod(m, KF // NW)
                    for k in range(KD):
                        nc.tensor.matmul(
                            pa,
                            lhsT=w1c[cw][:, k, mm * P:(mm + 1) * P],
                            rhs=hT[:, k, :],
                            start=(k == 0), stop=(k == KD - 1),
                        )
                    nc.scalar.activation(
                        aT[:, m, :], pa, AF.Relu,
                        bias=b1_t[:, m:m + 1], scale=1.0,
                    )

                # FF2 + gated residual combine, per token tile
                for ti in range(gt):
                    tt = t0 + ti
                    py = pyp.tile([P, D], f32, tag="py")
                    for k2 in range(KF):
                        cw, kk = divmod(k2, KF // NW)
                        lhs = aT[:, k2, ti * P:(ti + 1) * P]
                        nc.tensor.matmul(
                            py[:, 0:512], lhsT=lhs, rhs=w2c[cw][:, kk, 0:512],
                            start=(k2 == 0), stop=(k2 == KF - 1),
                        )
                        nc.tensor.matmul(
                            py[:, 512:1024], lhsT=lhs, rhs=w2c[cw][:, kk, 512:1024],
                            start=(k2 == 0), stop=(k2 == KF - 1),
                        )
                    o_t = outp.tile([P, D], f32, tag="o")
                    # o = alpha * ffn2
                    nc.scalar.activation(
                        o_t, py, AF.Copy, bias=0.0, scale=al_t[:, tt:tt + 1]
                    )
                    # o += xs   (residual; alpha*y + (1-alpha)*xs == xs + alpha*ffn2)
                    nc.vector.tensor_add(o_t, o_t, xs_tiles[ti])
                    nc.sync.dma_start(out_d[tt * P:(tt + 1) * P, :], o_t)

    nc.compile()
    return nc


def _prepare(inputs):
    """Host routing + per-core input packing. Returns (in_maps, perm, meta)."""
    x = np.ascontiguousarray(
        np.asarray(inputs["input_features"], dtype=np.float32).reshape(-1, D)
    )
    cent = np.asarray(inputs["centroids"], np.float32)
    ln_g = np.asarray(inputs["ln_g"], np.float32)
    ln_b = np.asarray(inputs["ln_b"], np.float32)
    w1 = np.asarray(inputs["w1"], np.float32)
    b1 = np.asarray(inputs["b1"], np.float32)
    w2 = np.asarray(inputs["w2"], np.float32)

    assign, alpha = _routing(x, cent)
    counts = np.bincount(assign, minlength=E)
    order = np.argsort(assign, kind="stable")
    segs = np.concatenate([[0], np.cumsum(counts)])
    C = max(P, int(-(-int(counts.max()) // P) * P))
    nt = C // P

    bf = ml_dtypes.bfloat16
    in_maps = []
    perm = []
    for e in range(E):
        idx = order[segs[e]:segs[e + 1]]
        ne = len(idx)
        xs = np.zeros((C, D), np.float32)
        xs[:ne] = x[idx]
        al = np.zeros((C,), np.float32)
        al[:ne] = alpha[idx]
        alphap = np.ascontiguousarray(al.reshape(nt, P).T)
        w1te = np.ascontiguousarray((w1[e] * ln_g[e][None, :]).T.astype(bf))
        w2te = np.ascontiguousarray(w2[e].T.astype(bf))
        b1e = (b1[e] + ln_b[e] @ w1[e].T).astype(np.float32)
        b1p = np.ascontiguousarray(b1e.reshape(KF, P).T)
        in_maps.append(
            {"xs": xs, "alphap": alphap, "w1t": w1te, "w2t": w2te, "b1p": b1p}
        )
        perm.append(idx)
    return in_maps, perm, (C, alpha)


def _unshard(inputs, results, perm, alpha):
    b2 = np.asarray(inputs["b2"], np.float32)
    x_shape = np.asarray(inputs["input_features"]).shape
    T = x_shape[0] * x_shape[1]
    out = np.empty((T, D), np.float32)
    for e in range(E):
        idx = perm[e]
        oe = np.asarray(results[e]["out"][:len(idx)], np.float32)
        if np.any(b2[e]):
            oe = oe + alpha[idx][:, None] * b2[e][None, :]
        out[idx] = oe
    return out.reshape(x_shape)


def run(inputs, **spmd_kwargs):
    """Full pipeline; returns (output, BassKernelResults, nc)."""
    from concourse.bass_utils import run_bass_kernel_spmd

    in_maps, perm, (C, alpha) = _prepare(inputs)
    nc = _build(C)
    res = run_bass_kernel_spmd(nc, in_maps, core_ids=list(range(E)), **spmd_kwargs)
    out = _unshard(inputs, res.results, perm, alpha)
    return out, res, nc


def kernel(**inputs) -> np.ndarray:
    out, _, _ = run(inputs)
    return out


# revision 7
# speedup vs baseline: 1.1902x; 1.1902x over previous
"""Trainium2 Bass kernel for nn_BaseLayer (MoE routing, 8 experts).

Strategy (expert-parallel, per the sharding hint):
  * Host computes the router exactly as the reference does (token-expert
    affinities + argmax + sigmoid gate) with jax-on-CPU so the assignment
    bit-matches the reference, then sorts tokens by expert.  In Bass all
    collectives must be compile-time static, so the dynamic
    dispatch/combine (all_to_all with runtime split sizes) is realized by
    the host sharding step: core e receives expert e's tokens, padded to a
    common capacity C so that a single NEFF runs SPMD on all 8 cores.
  * Each core runs the heavy part on device: LayerNorm -> FF1(+bias,relu)
    -> FF2 -> residual + sigmoid-gated combine, with the expert's weights
    resident in SBUF as bf16 and all matmuls on the PE array.
  * ln_g / ln_b are folded into w1 / b1 on the host (exact for the actual
    inputs where ln_g=1, ln_b=0); b2 is applied on the host during
    unsharding (exact for the actual inputs where b2=0).

The output permutation is the inverse of the sort, so the final output is
independent of sort order; only the argmax assignment must match the
reference, which host-side jax-on-CPU guarantees.
"""

import numpy as np
import ml_dtypes

D = 1024   # embed dim
F = 4096   # ffn dim
E = 8      # experts == cores
P = 128    # partitions
KD = D // P        # 8  k-tiles over D
KF = F // P        # 32 k-tiles over F
GROUP_TILES = 2    # token tiles (of 128) processed per FF1 batch
NW = 4             # weight DMA chunks (consumption-ordered)
EPS = 1e-5


def _routing(x, centroids):
    """Affinity/argmax/alpha exactly like the reference (jax on CPU)."""
    try:
        import jax
        import jax.numpy as jnp

        cpu = jax.devices("cpu")[0]
        with jax.default_device(cpu):
            aff = jnp.asarray(x) @ jnp.asarray(centroids).T
            assign = jnp.argmax(aff, axis=1)
            alpha = jax.nn.sigmoid(
                jnp.take_along_axis(aff, assign[:, None], axis=1)
            )
            return np.asarray(assign), np.asarray(alpha)[:, 0].astype(np.float32)
    except Exception:
        aff = x.astype(np.float32) @ centroids.astype(np.float32).T
        assign = np.argmax(aff, axis=1)
        sel = np.take_along_axis(aff, assign[:, None], axis=1)[:, 0]
        alpha = 1.0 / (1.0 + np.exp(-sel.astype(np.float64)))
        return assign, alpha.astype(np.float32)


def _build(C):
    """Build the per-core Bass program for capacity C (multiple of 128)."""
    import concourse.bacc as bacc
    import concourse.mybir as mybir
    import concourse.tile as tile
    from concourse.masks import make_identity

    f32 = mybir.dt.float32
    bf16 = mybir.dt.bfloat16
    AF = mybir.ActivationFunctionType
    ALU = mybir.AluOpType

    nt = C // P
    groups = []
    t = 0
    while t < nt:
        g = min(GROUP_TILES, nt - t)
        groups.append((t, g))
        t += g

    nc = bacc.Bacc("TRN2", target_bir_lowering=False, debug=False)
    xs_d = nc.dram_tensor("xs", [C, D], f32, kind="ExternalInput").ap()
    al_d = nc.dram_tensor("alphap", [P, nt], f32, kind="ExternalInput").ap()
    w1_d = nc.dram_tensor("w1t", [D, F], bf16, kind="ExternalInput").ap()
    w2_d = nc.dram_tensor("w2t", [F, D], bf16, kind="ExternalInput").ap()
    b1_d = nc.dram_tensor("b1p", [P, KF], f32, kind="ExternalInput").ap()
    out_d = nc.dram_tensor("out", [C, D], f32, kind="ExternalOutput").ap()

    with tile.TileContext(nc) as tc:
        with (
            tc.tile_pool(name="wpool", bufs=1) as wpool,
            tc.tile_pool(name="consts", bufs=1) as consts,
            tc.tile_pool(name="xsp", bufs=6) as xsp,
            tc.tile_pool(name="hp", bufs=2) as hp,
            tc.tile_pool(name="hTp", bufs=2) as hTp,
            tc.tile_pool(name="aTp", bufs=1) as aTp,
            tc.tile_pool(name="statp", bufs=3) as statp,
            tc.tile_pool(name="outp", bufs=2) as outp,
            tc.tile_pool(name="ptrp", bufs=2, space="PSUM") as ptrp,
            tc.tile_pool(name="pap", bufs=2, space="PSUM") as pap,
            tc.tile_pool(name="pyp", bufs=2, space="PSUM") as pyp,
        ):
            ident = consts.tile([P, P], bf16)
            make_identity(nc, ident)
            eps_t = consts.tile([P, 1], f32)
            nc.vector.memset(eps_t, EPS)
            b1_t = consts.tile([P, KF], f32)
            nc.sync.dma_start(b1_t, b1_d)
            al_t = consts.tile([P, nt], f32)
            nc.sync.dma_start(al_t, al_d)

            def emit_ln(tt, hT, ti):
                """DMA a token tile, layernorm it, transpose into hT."""
                xs_t = xsp.tile([P, D], f32, tag="xs", name=f"xs{tt}")
                nc.sync.dma_start(xs_t, xs_d[tt * P:(tt + 1) * P, :])
                st = statp.tile([P, 2, 6], f32, tag="st")
                nc.vector.bn_stats(st[:, 0, :], xs_t[:, 0:512])
                nc.vector.bn_stats(st[:, 1, :], xs_t[:, 512:1024])
                mv = statp.tile([P, 2], f32, tag="mv")
                nc.vector.bn_aggr(mv, st)
                # mv[:,1] := 1/sqrt(var+eps)
                nc.scalar.activation(
                    mv[:, 1:2], mv[:, 1:2], AF.Sqrt,
                    bias=eps_t[:, 0:1], scale=1.0,
                )
                nc.vector.reciprocal(mv[:, 1:2], mv[:, 1:2])
                h_t = hp.tile([P, D], bf16, tag="h")
                # h = (x - mean) * rstd
                nc.vector.tensor_scalar(
                    out=h_t, in0=xs_t,
                    scalar1=mv[:, 0:1], scalar2=mv[:, 1:2],
                    op0=ALU.subtract, op1=ALU.mult,
                )
                ptr = ptrp.tile([P, KD, P], bf16, tag="ptr")
                for k in range(KD):
                    nc.tensor.transpose(
                        ptr[:, k, :], h_t[:, k * P:(k + 1) * P], ident
                    )
                nc.vector.tensor_copy(hT[:, :, ti * P:(ti + 1) * P], ptr)
                return xs_t

            # Front-load the token DMA + layernorm + transpose for the first
            # PRE groups so their DMAs sit ahead of the bulk weight load in
            # the queues; PE can then start FF1 as soon as w1 chunk 0 lands.
            PRE = min(2, len(groups))
            pre_state = []
            for (t0, gt) in groups[:PRE]:
                hT = hTp.tile([P, KD, gt * P], bf16, tag="hT")
                xs_tiles = [emit_ln(t0 + ti, hT, ti) for ti in range(gt)]
                pre_state.append((hT, xs_tiles))

            # Expert weights, resident in SBUF as bf16, DMA'd in NW chunks
            # ordered to match first-group consumption order.
            w1c = []
            w2c = []
            for c in range(NW):
                fw = F // NW
                t1 = wpool.tile([P, KD, fw], bf16, name=f"w1c{c}", tag=f"w1c{c}")
                t2 = wpool.tile([P, KF // NW, D], bf16, name=f"w2c{c}", tag=f"w2c{c}")
                for k in range(KD):
                    nc.sync.dma_start(
                        t1[:, k, :], w1_d[k * P:(k + 1) * P, c * fw:(c + 1) * fw]
                    )
                for kk in range(KF // NW):
                    k2 = c * (KF // NW) + kk
                    nc.sync.dma_start(t2[:, kk, :], w2_d[k2 * P:(k2 + 1) * P, :])
                w1c.append(t1)
                w2c.append(t2)

            for gi, (t0, gt) in enumerate(groups):
                n = gt * P
                if gi < PRE:
                    hT, xs_tiles = pre_state[gi]
                else:
                    hT = hTp.tile([P, KD, n], bf16, tag="hT")
                    xs_tiles = [emit_ln(t0 + ti, hT, ti) for ti in range(gt)]

                # FF1: aT[f, tok] = relu((h @ w1t).T + b1)
                aT = aTp.tile([P, KF, n], bf16, tag="aT")
                for m in range(KF):
                    pa = pap.tile([P, n], f32, tag="pa")
                    cw, mm = divmod(m, KF // NW)
                    for k in range(KD):
                        nc.tensor.matmul(
                            pa,
                            lhsT=w1c[cw][:, k, mm * P:(mm + 1) * P],
                            rhs=hT[:, k, :],
                            start=(k == 0), stop=(k == KD - 1),
                        )
                    nc.scalar.activation(
                        aT[:, m, :], pa, AF.Relu,
                        bias=b1_t[:, m:m + 1], scale=1.0,
                    )

                # FF2 + gated residual combine, per token tile
                for ti in range(gt):
                    tt = t0 + ti
                    py = pyp.tile([P, D], f32, tag="py")
                    for k2 in range(KF):
                        cw, kk = divmod(k2, KF // NW)
                        lhs = aT[:, k2, ti * P:(ti + 1) * P]
                        nc.tensor.matmul(
                            py[:, 0:512], lhsT=lhs, rhs=w2c[cw][:, kk, 0:512],
                            start=(k2 == 0), stop=(k2 == KF - 1),
                        )
                        nc.tensor.matmul(
                            py[:, 512:1024], lhsT=lhs, rhs=w2c[cw][:, kk, 512:1024],
                            start=(k2 == 0), stop=(k2 == KF - 1),
                        )
                    o_t = outp.tile([P, D], f32, tag="o")
                    # o = alpha * ffn2
                    nc.scalar.activation(
                        o_t, py, AF.Copy, bias=0.0, scale=al_t[:, tt:tt + 1]
                    )
                    # o += xs   (residual; alpha*y + (1-alpha)*xs == xs + alpha*ffn2)
                    nc.vector.tensor_add(o_t, o_t, xs_tiles[ti])
                    nc.sync.dma_start(out_d[tt * P:(tt + 1) * P, :], o_t)

    nc.compile()
    return nc


def _prepare(inputs):
    """Host routing + per-core input packing. Returns (in_maps, perm, meta)."""
    x = np.ascontiguousarray(
        np.asarray(inputs["input_features"], dtype=np.float32).reshape(-1, D)
    )
    cent = np.asarray(inputs["centroids"], np.float32)
    ln_g = np.asarray(inputs["ln_g"], np.float32)
    ln_b = np.asarray(inputs["ln_b"], np.float32)
    w1 = np.asarray(inputs["w1"], np.float32)
    b1 = np.asarray(inputs["b1"], np.float32)
    w2 = np.asarray(inputs["w2"], np.float32)

    assign, alpha = _routing(x, cent)
    counts = np.bincount(assign, minlength=E)
    order = np.argsort(assign, kind="stable")
    segs = np.concatenate([[0], np.cumsum(counts)])
    C = max(P, int(-(-int(counts.max()) // P) * P))
    nt = C // P

    bf = ml_dtypes.bfloat16
    in_maps = []
    perm = []
    for e in range(E):
        idx = order[segs[e]:segs[e + 1]]
        ne = len(idx)
        xs = np.zeros((C, D), np.float32)
        xs[:ne] = x[idx]
        al = np.zeros((C,), np.float32)
        al[:ne] = alpha[idx]
        alphap = np.ascontiguousarray(al.reshape(nt, P).T)
        w1te = np.ascontiguousarray((w1[e] * ln_g[e][None, :]).T.astype(bf))
        w2te = np.ascontiguousarray(w2[e].T.astype(bf))
        b1e = (b1[e] + ln_b[e] @ w1[e].T).astype(np.float32)
        b1p = np.ascontiguousarray(b1e.reshape(KF, P).T)
        in_maps.append(
            {"xs": xs, "alphap": alphap, "w1t": w1te, "w2t": w2te, "b1p": b1p}
        )
        perm.append(idx)
    return in_maps, perm, (C, alpha)


def _unshard(inputs, results, perm, alpha):
    b2 = np.asarray(inputs["b2"], np.float32)
    x_shape = np.asarray(inputs["input_features"]).shape
    T = x_shape[0] * x_shape[1]
    out = np.empty((T, D), np.float32)
    for e in range(E):
        idx = perm[e]
        oe = np.asarray(results[e]["out"][:len(idx)], np.float32)
        if np.any(b2[e]):
            oe = oe + alpha[idx][:, None] * b2[e][None, :]
        out[idx] = oe
    return out.reshape(x_shape)


def run(inputs, **spmd_kwargs):
    """Full pipeline; returns (output, BassKernelResults, nc)."""
    from concourse.bass_utils import run_bass_kernel_spmd

    in_maps, perm, (C, alpha) = _prepare(inputs)
    nc = _build(C)
    res = run_bass_kernel_spmd(nc, in_maps, core_ids=list(range(E)), **spmd_kwargs)
    out = _unshard(inputs, res.results, perm, alpha)
    return out, res, nc


def kernel(**inputs) -> np.ndarray:
    out, _, _ = run(inputs)
    return out
